# revision 1
# baseline (speedup 1.0000x reference)
"""Trainium2 Bass kernel for nn_Encoder_block (B=128,S=512,D=24,H=4,HD=6,DFF=48).

Strategy: pure data parallel over batch — 16 batches per NeuronCore x 8 cores.
Per core everything runs in "T-layout" ([d, token] with d on partitions),
processed in 4 groups of 4 batches banded onto the 128 partitions
(batch p of a group occupies partitions 32p..32p+24).

Key tricks:
  - QKV projection emits Q^T/K^T per head at partition bands 32h..32h+6 and
    V^T at rows 8:32 of the same PSUM tile, so scores matmuls can row-tile
    (tile_position=(32h,0)) straight out of one SBUF copy.
  - Scores are computed TRANSPOSED (S^T[k,q]) so softmax's sum over k is a
    partition reduce done for free by an extra ones-column in the AV lhsT.
  - AV is col-tiled 4-heads-per-bank; the softmax denominator rides along as
    lhsT column 6.  Normalization is one reciprocal + gpsimd partition
    broadcasts + one tensor_tensor multiply.
  - LayerNorm runs in T-layout: mean/E[y^2] via band-selector matmuls,
    rstd = exp(-0.5*ln(var+eps)) so the whole kernel uses ONE ACT table set
    (natural_log_exp_and_others) — no table thrash with the softmax exps.
  - Output leaves as 32x32 DVE block-transpose + strided DMA gather.
"""

import os
import sys

import numpy as np

for _p in ("/opt/trn_rl_repo", "/opt/trn_rl_repo/concourse"):
    if os.path.isdir(_p) and _p not in sys.path:
        sys.path.insert(0, _p)

import concourse.bass as bass
import concourse.bacc as bacc
import concourse.mybir as mybir
import concourse.tile as tile
from concourse.bass_utils import run_bass_kernel_spmd

F32 = mybir.dt.float32
BF16 = mybir.dt.bfloat16
BF16_ATTN = True
ADT = BF16 if BF16_ATTN else F32
AF = mybir.ActivationFunctionType
ALU = mybir.AluOpType

B, S, D = 128, 512, 24
H, HD, DFF = 4, 6, 48
EPS = 1e-5
NCORES = 8
NB = B // NCORES          # batches per core = 16
NGROUPS = NB // 4         # groups of 4 banded batches = 4
SCALE = 1.0 / np.sqrt(HD)  # folded into the exp


def _host_consts(Wq, Wk, Wv, Wo, W1, W2, g1, b1, g2, b2):
    """Pre-layout all weights on the host (numpy) into the banded SBUF forms
    the kernel wants.  All float32."""
    c = {}
    # mm1 lhsT (per band replicated): cols 32h+j (j<6) = Wq[6h+j, :],
    # cols 8..32 = Wv rows; result rows are Q^T bands + V^T block.
    wqk1 = np.zeros((D, 128), np.float32)
    wk2 = np.zeros((D, 128), np.float32)
    for h in range(H):
        for j in range(HD):
            wqk1[:, 32 * h + j] = Wq[6 * h + j, :]
            wk2[:, 32 * h + j] = Wk[6 * h + j, :]
    for dv in range(D):
        wqk1[:, 8 + dv] = Wv[dv, :]
    WQK1 = np.zeros((128, 128), np.float32)
    WK2 = np.zeros((128, 128), np.float32)
    for p in range(4):
        WQK1[32 * p : 32 * p + D, :] = wqk1
        WK2[32 * p : 32 * p + D, :] = wk2
    c["wqk1"] = WQK1
    c["wk2"] = WK2

    # Wo lhsT: rows 32h+1+j = Wo[:, 6h+j] (row 32h is the denominator slot)
    # 32 cols so psum rows 24:32 of each output band are written (zeros)
    WOE = np.zeros((128, 32), np.float32)
    for h in range(H):
        for j in range(HD):
            WOE[32 * h + 1 + j, 0:D] = Wo[:, 6 * h + j]
    c["woe"] = WOE

    # band selectors (M=128 so every psum row is written; outputs at rows 32p):
    # cb1 col 32p = -1/24 over band p (gives -mean), cb2 col 32p = +1/24 (E[y^2])
    CB1 = np.zeros((128, 128), np.float32)
    CB2 = np.zeros((128, 128), np.float32)
    for p in range(4):
        CB1[32 * p : 32 * p + D, 32 * p] = -1.0 / D
        CB2[32 * p : 32 * p + D, 32 * p] = 1.0 / D
    c["cb1"] = CB1
    c["cb2"] = CB2

    # FFN W1 lhsT: variant p picks band p: rows 32p+d, col 64p+m = W1[m, d]
    # (64 cols per variant so the full 64-row psum half gets written)
    W1E = np.zeros((128, 4 * 64), np.float32)
    for p in range(4):
        W1E[32 * p : 32 * p + D, 64 * p : 64 * p + DFF] = W1.T
    c["w1e"] = W1E

    # FFN W2 lhsT: even variant rows 0:48, odd variant rows 64:112
    # (32 cols per variant so full psum bands get written)
    W2E = np.zeros((128, 2 * 32), np.float32)
    W2E[0:DFF, 0:D] = W2.T
    W2E[64 : 64 + DFF, 32 : 32 + D] = W2.T
    c["w2e"] = W2E

    # identity for PE transposes
    c["idt"] = np.eye(128, dtype=np.float32)
    if BF16_ATTN:
        import ml_dtypes
        c["idtb"] = np.eye(32, dtype=ml_dtypes.bfloat16)
    else:
        c["idtb"] = np.eye(32, dtype=np.float32)

    # LN gains/biases banded as per-partition scalars [128, 4] = g1,b1,g2,b2
    GB = np.zeros((128, 4), np.float32)
    for p in range(4):
        GB[32 * p : 32 * p + D, 0] = g1
        GB[32 * p : 32 * p + D, 1] = b1
        GB[32 * p : 32 * p + D, 2] = g2
        GB[32 * p : 32 * p + D, 3] = b2
    c["gb"] = GB
    return c


CONST_SHAPES = {
    "wqk1": (128, 128),
    "wk2": (128, 128),
    "woe": (128, 32),
    "cb1": (128, 128),
    "cb2": (128, 128),
    "w1e": (128, 4 * 64),
    "w2e": (128, 2 * 32),
    "idt": (128, 128),
    "idtb": (32, 32),
    "gb": (128, 4),
}


def _pin_act_tables():
    """Force Exp and Ln to resolve to the combined natural_log_exp_and_others
    table set (otherwise the compiler ping-pongs exp_and_others <-> natural_log
    at every LayerNorm, ~1.3us per reload)."""
    import concourse.bacc as _bacc
    if getattr(_bacc, "_act_tables_pinned", False):
        return
    _orig = _bacc.get_activation_tables

    def _patched(arch):
        tables = dict(_orig(arch))
        keep = "natural_log_exp_and_others"
        for name in list(tables):
            if name != keep and (
                AF.Exp in tables[name] or AF.Ln in tables[name]
            ):
                tables[name] = set()
        return tables

    _bacc.get_activation_tables = _patched
    _bacc._act_tables_pinned = True


def build_nc(nb: int = NB) -> bass.Bass:
    """Build the per-core Bass program. nb = batches this core processes."""
    _pin_act_tables()
    ngroups = nb // 4
    nc = bacc.Bacc()
    x_in = nc.dram_tensor("x", [nb, S, D], F32, kind="ExternalInput")
    out = nc.dram_tensor("out", [nb, S, D], F32, kind="ExternalOutput")
    cin = {
        k: nc.dram_tensor(k, list(sh), ADT if k == "idtb" else F32, kind="ExternalInput")
        for k, sh in CONST_SHAPES.items()
    }

    with tile.TileContext(nc) as tc:
        import contextlib

        ctx = contextlib.ExitStack()
        with ctx:
            constp = ctx.enter_context(tc.tile_pool(name="consts", bufs=1))
            xnp = ctx.enter_context(tc.tile_pool(name="xn", bufs=2))
            xtp = ctx.enter_context(tc.tile_pool(name="xt", bufs=2))
            qkp = ctx.enter_context(tc.tile_pool(name="qk", bufs=2))
            vsbp = ctx.enter_context(tc.tile_pool(name="vsb", bufs=2))
            ep = ctx.enter_context(tc.tile_pool(name="e", bufs=2))
            rrp = ctx.enter_context(tc.tile_pool(name="rr", bufs=2))
            rbp = ctx.enter_context(tc.tile_pool(name="rb", bufs=2))
            otp = ctx.enter_context(tc.tile_pool(name="ot", bufs=2))
            y1p = ctx.enter_context(tc.tile_pool(name="y1", bufs=2))
            x1p = ctx.enter_context(tc.tile_pool(name="x1", bufs=2))
            hsp = ctx.enter_context(tc.tile_pool(name="hs", bufs=2))
            fsp = ctx.enter_context(tc.tile_pool(name="fs", bufs=2))
            ysqp = ctx.enter_context(tc.tile_pool(name="ysq", bufs=2))
            bcp = ctx.enter_context(tc.tile_pool(name="bc", bufs=4))
            smp = ctx.enter_context(tc.tile_pool(name="sm", bufs=8))
            ytp = ctx.enter_context(tc.tile_pool(name="yt", bufs=2))
            # PSUM: st(2) + qkv(2) + uo(2) + misc(2) = 8 banks
            stp = ctx.enter_context(tc.tile_pool(name="st", bufs=2, space="PSUM"))
            qkvp = ctx.enter_context(tc.tile_pool(name="qkv", bufs=2, space="PSUM"))
            uop = ctx.enter_context(tc.tile_pool(name="uo", bufs=2, space="PSUM"))
            miscp = ctx.enter_context(tc.tile_pool(name="mp", bufs=2, space="PSUM"))

            # ---- load constants ----
            C = {}
            for k, sh in CONST_SHAPES.items():
                dt = ADT if k == "idtb" else F32
                t = constp.tile(list(sh), dt, name=f"c_{k}")
                nc.sync.dma_start(out=t, in_=cin[k][:, :])
                C[k] = t
            eps_t = constp.tile([128, 1], F32, name="c_eps")
            nc.vector.memset(eps_t, EPS)

            def bcast_rows(dst, src_row):
                """Broadcast one SBUF row to a 32-row band via DMA with a
                step-0 free dim (gpsimd partition_broadcast is broken on HW)."""
                src_b = bass.AP(
                    tensor=src_row.tensor,
                    offset=src_row.offset,
                    ap=[list(src_row.ap[0]), [0, 32]] + [list(x) for x in src_row.ap[1:]],
                )
                nc.sync.dma_start(
                    out=dst.rearrange("p (x q) -> p x q", x=1), in_=src_b
                )

            def ln_block(Y, gcol, bcol, OUT):
                """LayerNorm over d (partition bands) of Y [128,512] in T-layout.
                gcol/bcol: [128,1] per-partition scalar APs. Writes OUT [128,512].
                """
                mps = miscp.tile([128, S], F32, name="mps", tag="mps")
                nc.tensor.matmul(
                    mps[:, :], C["cb1"][:, :], Y[:, :],
                    start=True, stop=True, tile_position=(0, 0),
                )
                MUN = smp.tile([128, S], F32, name="mun", tag="sm")
                nc.vector.tensor_copy(MUN[:, :], mps[:, :])
                YSQ = ysqp.tile([128, S], F32, name="ysq")
                nc.gpsimd.tensor_mul(YSQ[:, :], Y[:, :], Y[:, :])
                m2ps = miscp.tile([128, S], F32, name="m2ps", tag="mps")
                nc.tensor.matmul(
                    m2ps[:, :], C["cb2"][:, :], YSQ[:, :],
                    start=True, stop=True, tile_position=(0, 0),
                )
                MSQ = smp.tile([128, S], F32, name="msq", tag="sm")
                nc.gpsimd.tensor_mul(MSQ[:, :], MUN[:, :], MUN[:, :])
                VAR = smp.tile([128, S], F32, name="var", tag="sm")
                nc.vector.tensor_sub(VAR[:, :], m2ps[:, :], MSQ[:, :])
                LNV = smp.tile([128, S], F32, name="lnv", tag="sm")
                nc.scalar.activation(LNV[:, :], VAR[:, :], AF.Ln, bias=eps_t[:, :])
                RST = smp.tile([128, S], F32, name="rst", tag="sm")
                nc.scalar.activation(RST[:, :], LNV[:, :], AF.Exp, scale=-0.5)
                MUB = bcp.tile([128, S], F32, name="mub", tag="bc")
                RSB = bcp.tile([128, S], F32, name="rsb", tag="bc")
                for p in range(4):
                    bcast_rows(MUB[32 * p : 32 * p + 32, :], MUN[32 * p : 32 * p + 1, :])
                    bcast_rows(RSB[32 * p : 32 * p + 32, :], RST[32 * p : 32 * p + 1, :])
                nc.gpsimd.tensor_add(Y[:, :], Y[:, :], MUB[:, :])  # y - mu
                nc.gpsimd.tensor_mul(Y[:, :], Y[:, :], RSB[:, :])  # * rstd
                nc.vector.tensor_scalar(
                    OUT[:, :], Y[:, :], gcol, bcol, op0=ALU.mult, op1=ALU.add
                )

            for g in range(ngroups):
                XT4 = xtp.tile([128, S], F32, name="xt4")
                Y1 = y1p.tile([128, S], F32, name="y1")
                # x load: block layout U[32p+c, 32f+d] = x[b, 32f+c, d], then a
                # single 32x32 block transpose turns it into XT4 bands.
                U = xnp.tile([128, 16, 32], F32, name="xu")
                nc.vector.memset(U[:, :, D:32], 0.0)
                for p in range(4):
                    b = 4 * g + p
                    nc.sync.dma_start(
                        out=U[32 * p : 32 * p + 32, :, 0:D],
                        in_=x_in[b].rearrange("(f c) d -> c f d", c=32),
                    )
                nc.vector.transpose(XT4[:, :], U.rearrange("p a b -> p (a b)"))
                # ---------- per-batch attention ----------
                for p in range(4):
                    b = 4 * g + p
                    # QKV projections (row-tiled at band p)
                    QK = qkp.tile([128, 2 * S], ADT, name="qk")
                    ps1 = qkvp.tile([128, S], F32, name="ps1", tag="qkv")
                    nc.tensor.matmul(
                        ps1[:, :],
                        C["wqk1"][32 * p : 32 * p + D, :],
                        XT4[32 * p : 32 * p + D, :],
                        start=True, stop=True, tile_position=(32 * p, 0),
                    )
                    nc.vector.tensor_copy(QK[:, 0:S], ps1[:, :])
                    ps2 = qkvp.tile([128, S], F32, name="ps2", tag="qkv")
                    nc.tensor.matmul(
                        ps2[:, :],
                        C["wk2"][32 * p : 32 * p + D, :],
                        XT4[32 * p : 32 * p + D, :],
                        start=True, stop=True, tile_position=(32 * p, 0),
                    )
                    nc.vector.tensor_copy(QK[:, S : 2 * S], ps2[:, :])

                    # V: PE-transpose rows 0:32 of QK (V^T lives at rows 8:32)
                    psv = miscp.tile([128, 4 * 32], ADT, name="psv", tag="mps")
                    for t in range(4):
                        nc.tensor.transpose(
                            psv[:, 32 * t : 32 * (t + 1)],
                            QK[0:32, 128 * t : 128 * (t + 1)],
                            C["idtb"][:, :],
                        )
                    Vsb = vsbp.tile([128, 4, 4, 32], ADT, name="vsb")
                    nc.vector.memset(Vsb[:, :, :, :], 1.0)
                    nc.vector.tensor_copy(
                        Vsb[:, :, :, 1:7],
                        psv.rearrange("p (t x) -> p t x", t=4)[:, :, 8:32].rearrange(
                            "p t (h d) -> p t h d", d=6
                        ),
                    )

                    # scores (S^T) + exp, head-pairs share a 2-bank psum tile
                    E = ep.tile([128, 4, 4, S], ADT, name="e")
                    for t in range(4):
                        for h in range(4):
                            stt = stp.tile([128, S], F32, name="stt", tag="st")
                            nc.tensor.matmul(
                                stt[:, :],
                                QK[32 * h : 32 * h + HD, S + 128 * t : S + 128 * (t + 1)],
                                QK[32 * h : 32 * h + HD, 0:S],
                                start=True, stop=True,
                                tile_position=(32 * h, 0),
                            )
                            nc.scalar.activation(
                                E[:, t, h, :],
                                stt[:, :],
                                AF.Exp,
                                scale=float(SCALE),
                            )

                    # AV col-tiled by head; ones column -> denominators
                    UO = uop.tile([128, S], F32, name="uo")
                    for t in range(4):
                        for h in range(4):
                            nc.tensor.matmul(
                                UO[32 * h : 32 * h + 32, :],
                                Vsb[:, t, h, :],
                                E[:, t, h, :],
                                start=(t == 0), stop=(t == 3),
                                tile_position=(0, 32 * h),
                                skip_group_check=True,
                            )
                    RR = rrp.tile([128, S], F32, name="rrt")
                    nc.vector.reciprocal_approx_fast(RR[:, :], UO[:, :])
                    RB = rbp.tile([128, S], F32, name="rbt")
                    for h in range(4):
                        bcast_rows(RB[32 * h : 32 * h + 32, :], RR[32 * h : 32 * h + 1, :])
                    OTn = otp.tile([128, S], F32, name="otn")
                    nc.vector.tensor_mul(OTn[:, :], UO[:, :], RB[:, :])

                    # Wo projection -> band p of wo psum, then residual into Y1
                    wops = miscp.tile([32, S], F32, name="wops", tag="mps")
                    nc.tensor.matmul(
                        wops[:, :], C["woe"][:, :], OTn[:, :],
                        start=True, stop=True, tile_position=(0, 0),
                    )
                    nc.vector.tensor_add(
                        Y1[32 * p : 32 * p + 32, :],
                        wops[:, :],
                        XT4[32 * p : 32 * p + 32, :],
                    )

                # ---------- LN1 ----------
                X1 = x1p.tile([128, S], F32, name="x1")
                ln_block(Y1, C["gb"][:, 0:1], C["gb"][:, 1:2], X1)

                # ---------- FFN ----------
                F4s = None
                FS = fsp.tile([128, S], F32, name="fs")
                F4 = miscp.tile([128, S], F32, name="f4", tag="mps")
                for pair in range(2):
                    hps = miscp.tile([128, S], F32, name="hps", tag="mps")
                    for j in range(2):
                        p = 2 * pair + j
                        nc.tensor.matmul(
                            hps[64 * j : 64 * j + 64, :],
                            C["w1e"][:, 64 * p : 64 * (p + 1)],
                            X1[:, :],
                            start=True, stop=True, tile_position=(0, 64 * j),
                        )
                    HS = hsp.tile([128, S], F32, name="hs")
                    nc.vector.tensor_scalar_max(HS[:, :], hps[:, :], 0.0)
                    for j in range(2):
                        p = 2 * pair + j
                        nc.tensor.matmul(
                            F4[32 * p : 32 * p + 32, :],
                            C["w2e"][:, 32 * j : 32 * (j + 1)],
                            HS[:, :],
                            start=True, stop=True, tile_position=(0, 32 * p),
                            skip_group_check=True,
                        )
                nc.vector.tensor_scalar_max(FS[:, :], F4[:, :], 0.0)  # 2nd relu
                nc.gpsimd.tensor_add(FS[:, :], FS[:, :], X1[:, :])  # + x1

                # ---------- LN2 + output ----------
                Y2N = x1p.tile([128, S], F32, name="y2n", tag="x1b")
                ln_block(FS, C["gb"][:, 2:3], C["gb"][:, 3:4], Y2N)
                Y2T = ytp.tile([128, S], F32, name="y2t")
                nc.vector.transpose(Y2T[:, :], Y2N[:, :])
                for p in range(4):
                    b = 4 * g + p
                    nc.sync.dma_start(
                        out=out[b].rearrange("(f r) d -> r f d", r=32),
                        in_=Y2T[32 * p : 32 * p + 32, :].rearrange(
                            "r (f c) -> r f c", c=32
                        )[:, :, 0:D],
                    )
    nc.compile()
    return nc


_NC_CACHE: dict[int, bass.Bass] = {}


def _get_nc(nb: int) -> bass.Bass:
    if nb not in _NC_CACHE:
        _NC_CACHE[nb] = build_nc(nb)
    return _NC_CACHE[nb]


def kernel(x, Wq, Wk, Wv, Wo, W1, W2, g1, b1, g2, b2):
    x = np.asarray(x, np.float32)
    consts = _host_consts(
        *(np.asarray(a, np.float32) for a in (Wq, Wk, Wv, Wo, W1, W2, g1, b1, g2, b2))
    )
    nc = _get_nc(NB)
    in_maps = []
    for c in range(NCORES):
        m = {"x": np.ascontiguousarray(x[c * NB : (c + 1) * NB])}
        m.update(consts)
        in_maps.append(m)
    res = run_bass_kernel_spmd(nc, in_maps, list(range(NCORES)))
    return np.concatenate([r["out"] for r in res.results], axis=0)



# revision 12
# speedup vs baseline: 1.2921x; 1.2921x over previous
"""Trainium2 Bass kernel for nn_Encoder_block (B=128,S=512,D=24,H=4,HD=6,DFF=48).

Pure data parallel over batch: 16 batches/core x 8 cores. Per core, batches
run in 4 groups of 4 banded onto the 128 partitions in T-layout ([d, token],
batch p of a group at partitions 32p..32p+24).

v2 design (vs the 489us baseline):
  - softmax exp is split across TWO engines: ScalarE native Exp and a custom
    DVE op EXP64_ANT computing (1 + s/64)^64 in ONE pass (6 chained
    squarings; max rel err ~0.7% on the score range, renormalized away by
    softmax).  Baseline ran all exp on ScalarE = hard 110us floor.
  - all large matmuls are bf16 (2x PE stream rate vs fp32), merged into
    [128,1024] PSUM tiles so exp/copy calls amortize instruction overhead.
  - V is projected in natural [token, d] layout (4 tiny N=32 matmuls)
    instead of PE-transposing V^T; kills 4x ~275ns transposes per batch.
  - Wo outputs for the 4 batches of a group are col-tiled into ONE psum bank
    so the residual add is a single [128,512] pass per group.
  - broadcasts (softmax denom, LN mu/rstd) are single multi-band DMAs.
  - group tails (LN1/FFN/LN2) are software-pipelined: stages of group g-1
    are emitted between the batches of group g so their dependency chains
    hide behind attention work.
"""

import os
import sys

import numpy as np

for _p in ("/opt/trn_rl_repo", "/opt/trn_rl_repo/concourse"):
    if os.path.isdir(_p) and _p not in sys.path:
        sys.path.insert(0, _p)

import concourse.bass as bass
import concourse.bacc as bacc
import concourse.mybir as mybir
import concourse.tile as tile
from concourse.bass_utils import run_bass_kernel_spmd

F32 = mybir.dt.float32
BF16 = mybir.dt.bfloat16
AF = mybir.ActivationFunctionType
ALU = mybir.AluOpType

B, S, D = 128, 512, 24
H, HD, DFF = 4, 6, 48
EPS = 1e-5
NCORES = 8
NB = B // NCORES          # batches per core = 16
SCALE = 1.0 / np.sqrt(HD)
EXPN = 64                 # exp(x) ~ (1 + x/EXPN)^EXPN, 6 squarings
C0EXP = float(SCALE / EXPN)

# per (batch, t, slot) exp engine: slot0 = heads 0/1, slot1 = heads 2/3.
# "A" = ScalarE native exp, "D" = VectorE EXP64_ANT.  5A/3D balances the
# engines once their other work is accounted for.
EXP_ASSIGN = ["A", "D", "D", "A", "A", "D", "A", "A"]


# --------------------------------------------------------------------------
# custom DVE ops (registered into concourse.dve_ops at import; the uop table
# is generated per-NEFF so no firmware change is involved)
# --------------------------------------------------------------------------
def _register_custom_dve_ops():
    import concourse.dve_ops as dve_ops
    from concourse.dve_spec import Spec, Src0, Src1, One, C0, sq, relu, lower, _has_src1
    from concourse.dve_uop import DveOpSpec

    if getattr(dve_ops, "_ant_encoder_ops", None) is not None:
        return dve_ops._ant_encoder_ops

    def _exp64_ref(in0, in1, s0, s1, imm2):
        return ((1.0 + in0.astype(np.float32) * s0) ** 64).astype(np.float32)

    b = One + Src0 * C0
    for _ in range(6):
        b = sq(b)
    specs = {
        "EXP64_ANT": Spec(body=b, reference=_exp64_ref),
        "RELU_ADD_ANT": Spec(
            body=relu(Src0) + Src1,
            reference=lambda in0, in1, s0, s1, imm2: np.maximum(
                np.nan_to_num(in0.astype(np.float32), nan=0.0), 0
            )
            + in1,
        ),
        "SUBSQ_ANT": Spec(
            body=Src0 - sq(Src1),
            reference=lambda in0, in1, s0, s1, imm2: in0.astype(np.float32)
            - in1.astype(np.float32) * in1,
        ),
    }
    ops = {}
    for name, spec in specs.items():
        shas = {}
        for ver in ("v3", "v4"):
            tmp = DveOpSpec(
                name=name, opcode=0, uops=lower(spec, ver=ver), rd1_en=_has_src1(spec)
            )
            shas[ver] = tmp.sha(ver)
        op = dve_ops.DveOp(name, spec, subdim=False, uops_sha=shas)
        dve_ops.OPS.append(op)
        ops[name] = op
    # re-derive the name->row map (rows must stay < 0x20)
    dve_ops._SUB_OPCODE_FOR_NAME.clear()
    dve_ops._SUB_OPCODE_FOR_NAME.update(
        {op.name: dve_ops._CUSTOM_DVE_ROW_BASE + i for i, op in enumerate(dve_ops.OPS)}
    )
    assert max(dve_ops._SUB_OPCODE_FOR_NAME.values()) < 0x20
    dve_ops.CUSTOM_DVE_SPECS.update({n: s for n, s in specs.items()})
    dve_ops._ant_encoder_ops = ops
    return ops


def _host_consts(Wq, Wk, Wv, Wo, W1, W2, g1, b1, g2, b2):
    """Pre-layout weights on the host into banded SBUF forms."""
    import ml_dtypes

    bf = ml_dtypes.bfloat16
    c = {}
    wqe = np.zeros((128, 128), np.float32)
    wke = np.zeros((128, 128), np.float32)
    for p in range(4):
        for h in range(H):
            for j in range(HD):
                wqe[32 * p : 32 * p + D, 32 * h + j] = Wq[6 * h + j, :]
                wke[32 * p : 32 * p + D, 32 * h + j] = Wk[6 * h + j, :]
    c["wqe"] = wqe.astype(bf)
    c["wke"] = wke.astype(bf)

    # V natural-layout projection: out[tok, j] = sum_d x[tok,d] * Wv[j, d]
    wve = np.zeros((128, 32), np.float32)
    for p in range(4):
        for j in range(D):
            wve[32 * p : 32 * p + D, j] = Wv[j, :]
    c["wve"] = wve.astype(bf)

    # Wo lhsT: rows 32h+1+j = Wo[:, 6h+j] (row 32h is the denominator slot)
    woe = np.zeros((128, 32), np.float32)
    for h in range(H):
        for j in range(HD):
            woe[32 * h + 1 + j, 0:D] = Wo[:, 6 * h + j]
    c["woe"] = woe.astype(bf)

    # band selectors: cb1 col 32p = -1/24 over band p (-mean), cb2 = +1/24
    cb1 = np.zeros((128, 128), np.float32)
    cb2 = np.zeros((128, 128), np.float32)
    for p in range(4):
        cb1[32 * p : 32 * p + D, 32 * p] = -1.0 / D
        cb2[32 * p : 32 * p + D, 32 * p] = 1.0 / D
    c["cb1"] = cb1
    c["cb2"] = cb2

    w1e = np.zeros((128, 4 * 64), np.float32)
    for p in range(4):
        w1e[32 * p : 32 * p + D, 64 * p : 64 * p + DFF] = W1.T
    c["w1e"] = w1e.astype(bf)

    w2e = np.zeros((128, 2 * 32), np.float32)
    w2e[0:DFF, 0:D] = W2.T
    w2e[64 : 64 + DFF, 32 : 32 + D] = W2.T
    c["w2e"] = w2e.astype(bf)

    gb = np.zeros((128, 4), np.float32)
    for p in range(4):
        gb[32 * p : 32 * p + D, 0] = g1
        gb[32 * p : 32 * p + D, 1] = b1
        gb[32 * p : 32 * p + D, 2] = g2
        gb[32 * p : 32 * p + D, 3] = b2
    c["gb"] = gb
    return c


CONST_SHAPES = {
    "wqe": ((128, 128), BF16),
    "wke": ((128, 128), BF16),
    "wve": ((128, 32), BF16),
    "woe": ((128, 32), BF16),
    "cb1": ((128, 128), F32),
    "cb2": ((128, 128), F32),
    "w1e": ((128, 4 * 64), BF16),
    "w2e": ((128, 2 * 32), BF16),
    "gb": ((128, 4), F32),
}


def _pin_act_tables():
    """Force Exp and Ln onto the combined natural_log_exp_and_others set so
    the compiler never thrashes ACT tables between softmax and LayerNorm."""
    import concourse.bacc as _bacc

    if getattr(_bacc, "_act_tables_pinned", False):
        return
    _orig = _bacc.get_activation_tables

    def _patched(arch):
        tables = dict(_orig(arch))
        keep = "natural_log_exp_and_others"
        for name in list(tables):
            if name != keep and (AF.Exp in tables[name] or AF.Ln in tables[name]):
                tables[name] = set()
        return tables

    _bacc.get_activation_tables = _patched
    _bacc._act_tables_pinned = True


def build_nc(nb: int = NB) -> bass.Bass:
    _pin_act_tables()
    OPS = _register_custom_dve_ops()
    EXP64, RELU_ADD, SUBSQ = OPS["EXP64_ANT"], OPS["RELU_ADD_ANT"], OPS["SUBSQ_ANT"]
    ngroups = nb // 4
    nc = bacc.Bacc()
    x_in = nc.dram_tensor("x", [nb, S, D], F32, kind="ExternalInput")
    out = nc.dram_tensor("out", [nb, S, D], F32, kind="ExternalOutput")
    cin = {
        k: nc.dram_tensor(k, list(sh), dt, kind="ExternalInput")
        for k, (sh, dt) in CONST_SHAPES.items()
    }

    with tile.TileContext(nc) as tc:
        import contextlib

        ctx = contextlib.ExitStack()
        with ctx:
            constp = ctx.enter_context(tc.tile_pool(name="consts", bufs=1))
            persp = ctx.enter_context(tc.tile_pool(name="pers", bufs=1))
            xtp = ctx.enter_context(tc.tile_pool(name="xt", bufs=2))
            xbp = ctx.enter_context(tc.tile_pool(name="xb", bufs=2))
            qkbp = ctx.enter_context(tc.tile_pool(name="qkb", bufs=2))
            ep = ctx.enter_context(tc.tile_pool(name="e", bufs=2))
            rrp = ctx.enter_context(tc.tile_pool(name="rr", bufs=2))
            rbp = ctx.enter_context(tc.tile_pool(name="rb", bufs=2))
            otp = ctx.enter_context(tc.tile_pool(name="ot", bufs=2))
            y1p = ctx.enter_context(tc.tile_pool(name="y1", bufs=2))
            ysqp = ctx.enter_context(tc.tile_pool(name="ysq", bufs=2))
            smp = ctx.enter_context(tc.tile_pool(name="sm", bufs=8))
            bcp = ctx.enter_context(tc.tile_pool(name="bc", bufs=4))
            x1p = ctx.enter_context(tc.tile_pool(name="x1", bufs=2))
            hsp = ctx.enter_context(tc.tile_pool(name="hs", bufs=4))
            fsp = ctx.enter_context(tc.tile_pool(name="fs", bufs=2))
            y2p = ctx.enter_context(tc.tile_pool(name="y2", bufs=2))
            # PSUM: scores 2x[128,1024] = 4 banks, UO 1, WOPS 1, misc 2
            scp = ctx.enter_context(tc.tile_pool(name="sc", bufs=2, space="PSUM"))
            uop = ctx.enter_context(tc.tile_pool(name="uo", bufs=1, space="PSUM"))
            wop = ctx.enter_context(tc.tile_pool(name="wo", bufs=1, space="PSUM"))
            mpp = ctx.enter_context(tc.tile_pool(name="mp", bufs=2, space="PSUM"))

            # ---- constants ----
            C = {}
            for k, (sh, dt) in CONST_SHAPES.items():
                t = constp.tile(list(sh), dt, name=f"c_{k}")
                nc.sync.dma_start(out=t, in_=cin[k][:, :])
                C[k] = t
            eps_t = constp.tile([128, 1], F32, name="c_eps")
            nc.vector.memset(eps_t, EPS)

            # persistent tiles: input-stage U (zero-padded cols) and Vsb
            # (ones outside the V slots), double-buffered manually.
            U2 = []
            VSB2 = []
            for i in range(2):
                u = persp.tile([128, 16, 32], F32, name=f"u{i}")
                nc.vector.memset(u[:, :, D:32], 0.0)
                U2.append(u)
                v = persp.tile([128, 4, 128], BF16, name=f"vsb{i}")
                nc.gpsimd.memset(v[:, :, :], 1.0)
                VSB2.append(v)

            def bcast4(dst, src, eng=None):
                """src rows {0,32,64,96} -> dst 32-row bands (4 DMAs)."""
                eng = eng or nc.sync
                for h in range(4):
                    row = src[32 * h : 32 * h + 1, :]
                    src_b = bass.AP(
                        tensor=row.tensor,
                        offset=row.offset,
                        ap=[list(row.ap[0]), [0, 32]]
                        + [list(x) for x in row.ap[1:]],
                    )
                    eng.dma_start(
                        out=dst[32 * h : 32 * h + 32, :].rearrange(
                            "p (x q) -> p x q", x=1
                        ),
                        in_=src_b,
                    )

            # ---------------- software-pipelined tail stages ----------------
            # state carried between emission points
            st = {}

            def ln_stats(Y, tag):
                """Y [128,512] f32 SBUF -> (MUN=-mu, RST=rstd) SBUF tiles."""
                YSQ = ysqp.tile([128, S], F32, name=f"ysq_{tag}", tag="ysq")
                nc.gpsimd.tensor_mul(YSQ[:, :], Y[:, :], Y[:, :])
                mps = mpp.tile([128, S], F32, name=f"mps_{tag}", tag="mp")
                nc.tensor.matmul(
                    mps[:, :], C["cb1"][:, :], Y[:, :],
                    start=True, stop=True, tile_position=(0, 0),
                )
                m2ps = mpp.tile([128, S], F32, name=f"m2ps_{tag}", tag="mp")
                nc.tensor.matmul(
                    m2ps[:, :], C["cb2"][:, :], YSQ[:, :],
                    start=True, stop=True, tile_position=(0, 0),
                )
                MUN = smp.tile([128, S], F32, name=f"mun_{tag}", tag="sm")
                nc.scalar.copy(MUN[:, :], mps[:, :])
                VAR = smp.tile([128, S], F32, name=f"var_{tag}", tag="sm")
                nc.vector._custom_dve(
                    SUBSQ, out=VAR[:, :], in0=m2ps[:, :], in1=MUN[:, :]
                )
                LNV = smp.tile([128, S], F32, name=f"lnv_{tag}", tag="sm")
                nc.scalar.activation(LNV[:, :], VAR[:, :], AF.Ln, bias=eps_t[:, :])
                RST = smp.tile([128, S], F32, name=f"rst_{tag}", tag="sm")
                nc.scalar.activation(RST[:, :], LNV[:, :], AF.Exp, scale=-0.5)
                return MUN, RST

            def ln_norm(Y, MUN, RST, OUT, gcol, bcol, tag):
                """(Y - mu) * rstd * g + b -> OUT (in-place mangles Y)."""
                MUB = bcp.tile([128, S], F32, name=f"mub_{tag}", tag="bc")
                bcast4(MUB, MUN)
                RSB = bcp.tile([128, S], F32, name=f"rsb_{tag}", tag="bc")
                bcast4(RSB, RST)
                nc.gpsimd.tensor_add(Y[:, :], Y[:, :], MUB[:, :])
                nc.gpsimd.tensor_mul(Y[:, :], Y[:, :], RSB[:, :])
                nc.vector.tensor_scalar(
                    OUT[:, :], Y[:, :], gcol, bcol, op0=ALU.mult, op1=ALU.add
                )

            def tail_T0(g):
                """Y1 residual + LN1 stats for group g."""
                Y1 = y1p.tile([128, S], F32, name=f"y1_{g}", tag="y1")
                nc.vector.tensor_add(Y1[:, :], st["WOPS"][:, :], st["XT4"][:, :])
                st["Y1"] = Y1
                st["LN1"] = ln_stats(Y1, f"l1g{g}")

            def tail_T1(g):
                MUN, RST = st["LN1"]
                X1 = x1p.tile([128, S], BF16, name=f"x1_{g}", tag="x1")
                ln_norm(st["Y1"], MUN, RST, X1, C["gb"][:, 0:1], C["gb"][:, 1:2],
                        f"l1g{g}")
                st["X1"] = X1

            def tail_T2(g):
                X1 = st["X1"]
                F4 = None
                HS = []
                for pair in range(2):
                    hps = mpp.tile([128, S], F32, name=f"hps{pair}_{g}", tag="mp")
                    for j in range(2):
                        p4 = 2 * pair + j
                        nc.tensor.matmul(
                            hps[64 * j : 64 * j + 64, :],
                            C["w1e"][:, 64 * p4 : 64 * (p4 + 1)],
                            X1[:, :],
                            start=True, stop=True, tile_position=(0, 64 * j),
                        )
                    h = hsp.tile([128, S], BF16, name=f"hs{pair}_{g}", tag="hs")
                    nc.scalar.activation(h[:, :], hps[:, :], AF.Relu)
                    HS.append(h)
                F4 = mpp.tile([128, S], F32, name=f"f4_{g}", tag="mp")
                for pair in range(2):
                    for j in range(2):
                        p4 = 2 * pair + j
                        nc.tensor.matmul(
                            F4[32 * p4 : 32 * p4 + 32, :],
                            C["w2e"][:, 32 * j : 32 * (j + 1)],
                            HS[pair][:, :],
                            start=True, stop=True, tile_position=(0, 32 * p4),
                            skip_group_check=True,
                        )
                FS = fsp.tile([128, S], F32, name=f"fs_{g}", tag="fs")
                nc.vector._custom_dve(
                    RELU_ADD, out=FS[:, :], in0=F4[:, :], in1=X1[:, :]
                )
                st["FS"] = FS

            def tail_T3(g):
                FS = st["FS"]
                MUN, RST = ln_stats(FS, f"l2g{g}")
                Y2N = y2p.tile([128, S], F32, name=f"y2n_{g}", tag="y2n")
                ln_norm(FS, MUN, RST, Y2N, C["gb"][:, 2:3], C["gb"][:, 3:4],
                        f"l2g{g}")
                Y2T = y2p.tile([128, S], F32, name=f"y2t_{g}", tag="y2t")
                nc.vector.transpose(Y2T[:, :], Y2N[:, :])
                for pp in range(4):
                    nc.sync.dma_start(
                        out=out[4 * g + pp].rearrange("(f r) d -> r f d", r=32),
                        in_=Y2T[32 * pp : 32 * pp + 32, :].rearrange(
                            "r (f c) -> r f c", c=32
                        )[:, :, 0:D],
                    )

            TAILS = [tail_T0, tail_T1, tail_T2, tail_T3]

            # ------------------------- main loop -------------------------
            for b in range(nb):
                g, p = b // 4, b % 4
                if p == 0:
                    # group input: 1 DMA + block transpose + bf16 cast
                    U = U2[g % 2]
                    for pp in range(4):
                        nc.sync.dma_start(
                            out=U[32 * pp : 32 * pp + 32, :, 0:D],
                            in_=x_in[4 * g + pp].rearrange("(f c) d -> c f d", c=32),
                        )
                    XT4 = xtp.tile([128, S], F32, name=f"xt4_{g}", tag="xt")
                    nc.vector.transpose(XT4[:, :], U.rearrange("P a c -> P (a c)"))
                    XT4b = xbp.tile([128, S], BF16, name=f"xtb_{g}", tag="xb")
                    nc.vector.tensor_copy(XT4b[:, :], XT4[:, :])
                    st["XT4_new"], st["XT4b"] = XT4, XT4b

                XT4b = st["XT4b"]
                # Q/K projections -> bf16 SBUF
                ps_q = mpp.tile([128, S], F32, name=f"psq_{b}", tag="mp")
                nc.tensor.matmul(
                    ps_q[:, :], C["wqe"][32 * p : 32 * p + D, :],
                    XT4b[32 * p : 32 * p + D, :],
                    start=True, stop=True, tile_position=(32 * p, 0),
                )
                ps_k = mpp.tile([128, S], F32, name=f"psk_{b}", tag="mp")
                nc.tensor.matmul(
                    ps_k[:, :], C["wke"][32 * p : 32 * p + D, :],
                    XT4b[32 * p : 32 * p + D, :],
                    start=True, stop=True, tile_position=(32 * p, 0),
                )
                QKb = qkbp.tile([128, 2 * S], BF16, name=f"qkb_{b}", tag="qkb")
                nc.scalar.copy(QKb[:, 0:S], ps_q[:, :])
                nc.vector.tensor_copy(QKb[:, S : 2 * S], ps_k[:, :])

                # V natural layout: V4ps[:, 32c+j] = V[tok128c.., j]
                V4ps = mpp.tile([128, 128], F32, name=f"v4_{b}", tag="mp")
                for cch in range(4):
                    nc.tensor.matmul(
                        V4ps[:, 32 * cch : 32 * cch + 32],
                        XT4b[32 * p : 32 * p + D, 128 * cch : 128 * (cch + 1)],
                        C["wve"][32 * p : 32 * p + D, :],
                        start=True, stop=True, tile_position=(32 * p, 0),
                    )
                Vsb = VSB2[b % 2]
                nc.vector.tensor_copy(
                    Vsb.rearrange("P t (h m) -> P t h m", m=32)[:, :, :, 1 : 1 + HD],
                    V4ps.rearrange("P (c x) -> P c x", x=32)[:, :, 0:D].rearrange(
                        "P c (h m) -> P c h m", m=HD
                    ),
                )

                # scores + exp + AV
                E = ep.tile([128, 4, 4, S], BF16, name=f"e_{b}", tag="e")
                UO = uop.tile([128, S], F32, name=f"uo_{b}", tag="uo")
                for t in range(4):
                    for slot in range(2):
                        h0 = 2 * slot
                        SC = scp.tile([128, 2 * S], F32, name=f"sc{b}_{t}_{slot}",
                                      tag="sc")
                        for hh in range(2):
                            h = h0 + hh
                            nc.tensor.matmul(
                                SC[:, S * hh : S * (hh + 1)],
                                QKb[32 * h : 32 * h + HD,
                                    S + 128 * t : S + 128 * (t + 1)],
                                QKb[32 * h : 32 * h + HD, 0:S],
                                start=True, stop=True,
                                tile_position=(32 * h, 0),
                            )
                        edst = E[:, t, h0 : h0 + 2, :]
                        if EXP_ASSIGN[2 * t + slot] == "A":
                            nc.scalar.activation(
                                edst, SC[:, :], AF.Exp, scale=float(SCALE)
                            )
                        else:
                            nc.vector._custom_dve(
                                EXP64, out=edst, in0=SC[:, :], s0=C0EXP
                            )
                    for h in range(4):
                        nc.tensor.matmul(
                            UO[32 * h : 32 * h + 32, :],
                            Vsb[:, t, 32 * h : 32 * h + 32],
                            E[:, t, h, :],
                            start=(t == 0), stop=(t == 3),
                            tile_position=(0, 32 * h),
                            skip_group_check=True,
                        )

                # pipelined tail stage of the previous group: emitted after
                # this batch's PE-heavy attention so the tail's matmuls (cb/
                # ffn) queue behind AV and their cross-engine deps are ready.
                if g > 0:
                    TAILS[p](g - 1)

                # softmax denominator + normalize + Wo
                RR = rrp.tile([128, S], F32, name=f"rr_{b}", tag="rr")
                nc.vector.reciprocal_approx_fast(RR[:, :], UO[:, :])
                RB = rbp.tile([128, S], F32, name=f"rb_{b}", tag="rb")
                bcast4(RB, RR)
                OTn = otp.tile([128, S], BF16, name=f"ot_{b}", tag="ot")
                nc.vector.tensor_mul(OTn[:, :], UO[:, :], RB[:, :])
                if p == 0:
                    st["WOPS_new"] = wop.tile([128, S], F32, name=f"wops_{g}",
                                              tag="wops")
                nc.tensor.matmul(
                    st["WOPS_new"][32 * p : 32 * p + 32, :],
                    C["woe"][:, :], OTn[:, :],
                    start=True, stop=True, tile_position=(0, 32 * p),
                    skip_group_check=True,
                )
                if p == 3:
                    st["WOPS"], st["XT4"] = st["WOPS_new"], st["XT4_new"]

            # drain the last group's tail
            for stage in range(4):
                TAILS[stage](ngroups - 1)
    nc.compile()
    return nc


_NC_CACHE: dict[int, bass.Bass] = {}


def _get_nc(nb: int) -> bass.Bass:
    if nb not in _NC_CACHE:
        _NC_CACHE[nb] = build_nc(nb)
    return _NC_CACHE[nb]


def kernel(x, Wq, Wk, Wv, Wo, W1, W2, g1, b1, g2, b2):
    x = np.asarray(x, np.float32)
    consts = _host_consts(
        *(np.asarray(a, np.float32) for a in (Wq, Wk, Wv, Wo, W1, W2, g1, b1, g2, b2))
    )
    nc = _get_nc(NB)
    in_maps = []
    for c in range(NCORES):
        m = {"x": np.ascontiguousarray(x[c * NB : (c + 1) * NB])}
        m.update(consts)
        in_maps.append(m)
    res = run_bass_kernel_spmd(nc, in_maps, list(range(NCORES)))
    return np.concatenate([r["out"] for r in res.results], axis=0)


# revision 15
# speedup vs baseline: 1.6614x; 1.2858x over previous
"""Trainium2 Bass kernel for nn_Encoder_block (B=128,S=512,D=24,H=4,HD=6,DFF=48).

Pure data parallel over batch: 16 batches/core x 8 cores. Per core, batches
run in 4 groups of 4 banded onto the 128 partitions in T-layout ([d, token],
batch p of a group at partitions 32p..32p+24).

v3 design:
  - softmax exp split across ScalarE (native Exp) and VectorE (custom
    EXP64_ANT = (1+s/64)^64 in one fused pass); per-(t,head-pair) [128,1024]
    PSUM tiles amortize instruction overhead.
  - all matmuls bf16 (2x PE stream rate); LN selector matmuls run on bf16
    copies of y / y^2.
  - softmax denominator broadcast via ONE VectorE stream_shuffle (within-
    quadrant row-0 broadcast) instead of 4 DMAs - the SP DMA queue was a
    serializer.  LN mu/rstd ride a combined [128,1024] bf16 stats tile
    broadcast by 4 band DMAs per LN.
  - V projected in natural [token,d] layout (4 tiny N=32 matmuls).
  - Wo outputs col-tiled per group into one psum bank; residual add is one
    [128,512] pass per group.
  - group tails (LN1/FFN/LN2/out) split into 8 pipeline slots spread over
    the following TWO groups' batches, so every cross-engine dependency is
    ~one batch stale by the time its consumer is reached.
  - when g1==1,b1==0,g2==1,b2==0 (the spec's inputs), the LN affine is
    folded into the gp multiply (detected on host, no affine passes).
"""

import os
import sys

import numpy as np

for _p in ("/opt/trn_rl_repo", "/opt/trn_rl_repo/concourse"):
    if os.path.isdir(_p) and _p not in sys.path:
        sys.path.insert(0, _p)

import concourse.bass as bass
import concourse.bacc as bacc
import concourse.mybir as mybir
import concourse.tile as tile
from concourse.bass_utils import run_bass_kernel_spmd

F32 = mybir.dt.float32
BF16 = mybir.dt.bfloat16
AF = mybir.ActivationFunctionType
ALU = mybir.AluOpType

B, S, D = 128, 512, 24
H, HD, DFF = 4, 6, 48
EPS = 1e-5
NCORES = 8
NB = B // NCORES          # batches per core = 16
SCALE = 1.0 / np.sqrt(HD)
EXPN = 64                 # exp(x) ~ (1 + x/EXPN)^EXPN, 6 squarings
C0EXP = float(SCALE / EXPN)

# per (t, slot) exp engine: slot0 = heads 0/1, slot1 = heads 2/3.
EXP_ASSIGN = ["A", "D", "D", "A", "A", "D", "A", "D"]
BCAST_MASK = [0] * 32     # stream_shuffle: every quadrant row <- row 0


def _register_custom_dve_ops():
    import concourse.dve_ops as dve_ops
    from concourse.dve_spec import Spec, Src0, Src1, One, C0, sq, relu, lower, _has_src1
    from concourse.dve_uop import DveOpSpec

    if getattr(dve_ops, "_ant_encoder_ops", None) is not None:
        return dve_ops._ant_encoder_ops

    def _exp64_ref(in0, in1, s0, s1, imm2):
        return ((1.0 + in0.astype(np.float32) * s0) ** 64).astype(np.float32)

    b = One + Src0 * C0
    for _ in range(6):
        b = sq(b)
    specs = {
        "EXP64_ANT": Spec(body=b, reference=_exp64_ref),
        "RELU_ADD_ANT": Spec(
            body=relu(Src0) + Src1,
            reference=lambda in0, in1, s0, s1, imm2: np.maximum(
                np.nan_to_num(in0.astype(np.float32), nan=0.0), 0
            )
            + in1,
        ),
        "SUBSQ_ANT": Spec(
            body=Src0 - sq(Src1),
            reference=lambda in0, in1, s0, s1, imm2: in0.astype(np.float32)
            - in1.astype(np.float32) * in1.astype(np.float32),
        ),
    }
    ops = {}
    for name, spec in specs.items():
        shas = {}
        for ver in ("v3", "v4"):
            tmp = DveOpSpec(
                name=name, opcode=0, uops=lower(spec, ver=ver), rd1_en=_has_src1(spec)
            )
            shas[ver] = tmp.sha(ver)
        op = dve_ops.DveOp(name, spec, subdim=False, uops_sha=shas)
        dve_ops.OPS.append(op)
        ops[name] = op
    dve_ops._SUB_OPCODE_FOR_NAME.clear()
    dve_ops._SUB_OPCODE_FOR_NAME.update(
        {op.name: dve_ops._CUSTOM_DVE_ROW_BASE + i for i, op in enumerate(dve_ops.OPS)}
    )
    assert max(dve_ops._SUB_OPCODE_FOR_NAME.values()) < 0x20
    dve_ops.CUSTOM_DVE_SPECS.update({n: s for n, s in specs.items()})
    dve_ops._ant_encoder_ops = ops
    return ops


def _host_consts(Wq, Wk, Wv, Wo, W1, W2, g1, b1, g2, b2):
    import ml_dtypes

    bf = ml_dtypes.bfloat16
    c = {}
    wqe = np.zeros((128, 128), np.float32)
    wke = np.zeros((128, 128), np.float32)
    for p in range(4):
        for h in range(H):
            for j in range(HD):
                wqe[32 * p : 32 * p + D, 32 * h + j] = Wq[6 * h + j, :]
                wke[32 * p : 32 * p + D, 32 * h + j] = Wk[6 * h + j, :]
    c["wqe"] = wqe.astype(bf)
    c["wke"] = wke.astype(bf)

    wve = np.zeros((128, 32), np.float32)
    for p in range(4):
        for j in range(D):
            wve[32 * p : 32 * p + D, j] = Wv[j, :]
    c["wve"] = wve.astype(bf)

    woe = np.zeros((128, 32), np.float32)
    for h in range(H):
        for j in range(HD):
            woe[32 * h + 1 + j, 0:D] = Wo[:, 6 * h + j]
    c["woe"] = woe.astype(bf)

    cb1 = np.zeros((128, 128), np.float32)
    cb2 = np.zeros((128, 128), np.float32)
    for p in range(4):
        cb1[32 * p : 32 * p + D, 32 * p] = -1.0 / D
        cb2[32 * p : 32 * p + D, 32 * p] = 1.0 / D
    c["cb1"] = cb1.astype(bf)
    c["cb2"] = cb2.astype(bf)

    w1e = np.zeros((128, 4 * 64), np.float32)
    for p in range(4):
        w1e[32 * p : 32 * p + D, 64 * p : 64 * p + DFF] = W1.T
    c["w1e"] = w1e.astype(bf)

    w2e = np.zeros((128, 2 * 32), np.float32)
    w2e[0:DFF, 0:D] = W2.T
    w2e[64 : 64 + DFF, 32 : 32 + D] = W2.T
    c["w2e"] = w2e.astype(bf)

    gb = np.zeros((128, 4), np.float32)
    for p in range(4):
        gb[32 * p : 32 * p + D, 0] = g1
        gb[32 * p : 32 * p + D, 1] = b1
        gb[32 * p : 32 * p + D, 2] = g2
        gb[32 * p : 32 * p + D, 3] = b2
    c["gb"] = gb
    return c


CONST_SHAPES = {
    "wqe": ((128, 128), BF16),
    "wke": ((128, 128), BF16),
    "wve": ((128, 32), BF16),
    "woe": ((128, 32), BF16),
    "cb1": ((128, 128), BF16),
    "cb2": ((128, 128), BF16),
    "w1e": ((128, 4 * 64), BF16),
    "w2e": ((128, 2 * 32), BF16),
    "gb": ((128, 4), F32),
}


def _pin_act_tables():
    import concourse.bacc as _bacc

    if getattr(_bacc, "_act_tables_pinned", False):
        return
    _orig = _bacc.get_activation_tables

    def _patched(arch):
        tables = dict(_orig(arch))
        keep = "natural_log_exp_and_others"
        for name in list(tables):
            if name != keep and (AF.Exp in tables[name] or AF.Ln in tables[name]):
                tables[name] = set()
        return tables

    _bacc.get_activation_tables = _patched
    _bacc._act_tables_pinned = True


def build_nc(nb: int = NB, trivial_affine: bool = True) -> bass.Bass:
    _pin_act_tables()
    OPS = _register_custom_dve_ops()
    EXP64, RELU_ADD, SUBSQ = OPS["EXP64_ANT"], OPS["RELU_ADD_ANT"], OPS["SUBSQ_ANT"]
    ngroups = nb // 4
    nc = bacc.Bacc()
    x_in = nc.dram_tensor("x", [nb, S, D], F32, kind="ExternalInput")
    out = nc.dram_tensor("out", [nb, S, D], F32, kind="ExternalOutput")
    cin = {
        k: nc.dram_tensor(k, list(sh), dt, kind="ExternalInput")
        for k, (sh, dt) in CONST_SHAPES.items()
    }

    with tile.TileContext(nc) as tc:
        import contextlib

        ctx = contextlib.ExitStack()
        with ctx:
            constp = ctx.enter_context(tc.tile_pool(name="consts", bufs=1))
            persp = ctx.enter_context(tc.tile_pool(name="pers", bufs=1))
            xtp = ctx.enter_context(tc.tile_pool(name="xt", bufs=2))
            xbp = ctx.enter_context(tc.tile_pool(name="xb", bufs=2))
            qkbp = ctx.enter_context(tc.tile_pool(name="qkb", bufs=2))
            ep = ctx.enter_context(tc.tile_pool(name="e", bufs=2))
            rrp = ctx.enter_context(tc.tile_pool(name="rr", bufs=2))
            rbp = ctx.enter_context(tc.tile_pool(name="rb", bufs=2))
            otp = ctx.enter_context(tc.tile_pool(name="ot", bufs=2))
            y1p = ctx.enter_context(tc.tile_pool(name="y1", bufs=2))
            ybp = ctx.enter_context(tc.tile_pool(name="yb", bufs=4))
            smp = ctx.enter_context(tc.tile_pool(name="sm", bufs=4))
            bcp = ctx.enter_context(tc.tile_pool(name="bc", bufs=4))
            x1p = ctx.enter_context(tc.tile_pool(name="x1", bufs=2))
            hsp = ctx.enter_context(tc.tile_pool(name="hs", bufs=4))
            fsp = ctx.enter_context(tc.tile_pool(name="fs", bufs=2))
            y2p = ctx.enter_context(tc.tile_pool(name="y2", bufs=2))
            # PSUM: scores/qk 2x[128,1024]=4 banks, UO 1, WOPS 1, misc 2
            scp = ctx.enter_context(tc.tile_pool(name="sc", bufs=2, space="PSUM"))
            uop = ctx.enter_context(tc.tile_pool(name="uo", bufs=1, space="PSUM"))
            wop = ctx.enter_context(tc.tile_pool(name="wo", bufs=1, space="PSUM"))
            mpp = ctx.enter_context(tc.tile_pool(name="mp", bufs=2, space="PSUM"))

            C = {}
            for k, (sh, dt) in CONST_SHAPES.items():
                t = constp.tile(list(sh), dt, name=f"c_{k}")
                nc.sync.dma_start(out=t, in_=cin[k][:, :])
                C[k] = t
            eps_t = constp.tile([128, 1], F32, name="c_eps")
            nc.vector.memset(eps_t, EPS)

            U2, VSB2 = [], []
            for i in range(2):
                u = persp.tile([128, 16, 32], F32, name=f"u{i}")
                nc.vector.memset(u[:, :, D:32], 0.0)
                U2.append(u)
                v = persp.tile([128, 4, 128], BF16, name=f"vsb{i}")
                nc.gpsimd.memset(v[:, :, :], 1.0)
                VSB2.append(v)

            def bcast_dma(dst, src, ncols):
                """src rows {0,32,64,96} -> dst 32-row bands (4 DMAs, SP)."""
                for h in range(4):
                    row = src[32 * h : 32 * h + 1, :]
                    src_b = bass.AP(
                        tensor=row.tensor,
                        offset=row.offset,
                        ap=[list(row.ap[0]), [0, 32]]
                        + [list(x) for x in row.ap[1:]],
                    )
                    nc.sync.dma_start(
                        out=dst[32 * h : 32 * h + 32, :].rearrange(
                            "p (x q) -> p x q", x=1
                        ),
                        in_=src_b,
                    )

            # ------------- tail pipeline (8 slots over 2 groups) -------------
            st = {}

            def ln_statsA(Y, tag):
                """slot A: bf16 copies of y,y^2 + selector matmuls."""
                Yb = ybp.tile([128, S], BF16, name=f"yb_{tag}", tag="yb")
                nc.vector.tensor_copy(Yb[:, :], Y[:, :])
                YSQ = ybp.tile([128, S], BF16, name=f"ysq_{tag}", tag="yb")
                nc.gpsimd.tensor_mul(YSQ[:, :], Y[:, :], Y[:, :])
                mps = mpp.tile([128, S], F32, name=f"mps_{tag}", tag="mp")
                nc.tensor.matmul(
                    mps[:, :], C["cb1"][:, :], Yb[:, :],
                    start=True, stop=True, tile_position=(0, 0),
                )
                m2ps = mpp.tile([128, S], F32, name=f"m2ps_{tag}", tag="mp")
                nc.tensor.matmul(
                    m2ps[:, :], C["cb2"][:, :], YSQ[:, :],
                    start=True, stop=True, tile_position=(0, 0),
                )
                return mps, m2ps

            def ln_statsB(mm, tag):
                """slot B: -mu|rstd combined bf16 stats tile."""
                mps, m2ps = mm
                STT = smp.tile([128, 2 * S], BF16, name=f"stt_{tag}", tag="sm")
                nc.scalar.copy(STT[:, 0:S], mps[:, :])            # -mu (bf16)
                VAR = smp.tile([128, S], F32, name=f"var_{tag}", tag="var")
                nc.vector._custom_dve(
                    SUBSQ, out=VAR[:, :], in0=m2ps[:, :], in1=STT[:, 0:S]
                )
                LNV = smp.tile([128, S], F32, name=f"lnv_{tag}", tag="var")
                nc.scalar.activation(LNV[:, :], VAR[:, :], AF.Ln, bias=eps_t[:, :])
                nc.scalar.activation(STT[:, S : 2 * S], LNV[:, :], AF.Exp, scale=-0.5)
                return STT

            def ln_normC(Y, STT, OUT, gcol, bcol, tag):
                """slot C: broadcast + (y-mu)*rstd [*g+b] -> OUT."""
                BC = bcp.tile([128, 2 * S], BF16, name=f"bc_{tag}", tag="bc")
                bcast_dma(BC, STT, 2 * S)
                nc.gpsimd.tensor_add(Y[:, :], Y[:, :], BC[:, 0:S])
                if trivial_affine:
                    nc.gpsimd.tensor_mul(OUT[:, :], Y[:, :], BC[:, S : 2 * S])
                else:
                    nc.gpsimd.tensor_mul(Y[:, :], Y[:, :], BC[:, S : 2 * S])
                    nc.vector.tensor_scalar(
                        OUT[:, :], Y[:, :], gcol, bcol, op0=ALU.mult, op1=ALU.add
                    )

            def slot0(g):
                Y1 = y1p.tile([128, S], F32, name=f"y1_{g}", tag="y1")
                nc.vector.tensor_add(Y1[:, :], st["WOPS"][:, :], st["XT4"][:, :])
                st["Y1"] = Y1
                st["L1A"] = ln_statsA(Y1, f"l1g{g}")

            def slot1(g):
                st["L1B"] = ln_statsB(st["L1A"], f"l1g{g}")

            def slot2(g):
                X1 = x1p.tile([128, S], BF16, name=f"x1_{g}", tag="x1")
                ln_normC(st["Y1"], st["L1B"], X1,
                         C["gb"][:, 0:1], C["gb"][:, 1:2], f"l1g{g}")
                st["X1"] = X1

            def slot3(g):
                X1 = st["X1"]
                HS = []
                for pair in range(2):
                    hps = mpp.tile([128, S], F32, name=f"hps{pair}_{g}", tag="mp")
                    for j in range(2):
                        p4 = 2 * pair + j
                        nc.tensor.matmul(
                            hps[64 * j : 64 * j + 64, :],
                            C["w1e"][:, 64 * p4 : 64 * (p4 + 1)],
                            X1[:, :],
                            start=True, stop=True, tile_position=(0, 64 * j),
                        )
                    h = hsp.tile([128, S], BF16, name=f"hs{pair}_{g}", tag="hs")
                    nc.scalar.activation(h[:, :], hps[:, :], AF.Relu)
                    HS.append(h)
                F4 = mpp.tile([128, S], F32, name=f"f4_{g}", tag="mp")
                for pair in range(2):
                    for j in range(2):
                        p4 = 2 * pair + j
                        nc.tensor.matmul(
                            F4[32 * p4 : 32 * p4 + 32, :],
                            C["w2e"][:, 32 * j : 32 * (j + 1)],
                            HS[pair][:, :],
                            start=True, stop=True, tile_position=(0, 32 * p4),
                            skip_group_check=True,
                        )
                FS = fsp.tile([128, S], F32, name=f"fs_{g}", tag="fs")
                nc.vector._custom_dve(
                    RELU_ADD, out=FS[:, :], in0=F4[:, :], in1=X1[:, :]
                )
                st["FS"] = FS

            def slot4(g):
                st["L2A"] = ln_statsA(st["FS"], f"l2g{g}")

            def slot5(g):
                st["L2B"] = ln_statsB(st["L2A"], f"l2g{g}")

            def slot6(g):
                Y2N = y2p.tile([128, S], F32, name=f"y2n_{g}", tag="y2n")
                ln_normC(st["FS"], st["L2B"], Y2N,
                         C["gb"][:, 2:3], C["gb"][:, 3:4], f"l2g{g}")
                st["Y2N"] = Y2N

            def slot7(g):
                Y2T = y2p.tile([128, S], F32, name=f"y2t_{g}", tag="y2t")
                nc.vector.transpose(Y2T[:, :], st["Y2N"][:, :])
                for pp in range(4):
                    nc.sync.dma_start(
                        out=out[4 * g + pp].rearrange("(f r) d -> r f d", r=32),
                        in_=Y2T[32 * pp : 32 * pp + 32, :].rearrange(
                            "r (f c) -> r f c", c=32
                        )[:, :, 0:D],
                    )

            SLOTS = [slot0, slot1, slot2, slot3, slot4, slot5, slot6, slot7]

            def run_tails(gg, p):
                if gg >= 1:
                    SLOTS[p](gg - 1)
                if gg >= 2:
                    SLOTS[4 + p](gg - 2)

            # --------------------------- main loop ---------------------------
            for b in range(nb):
                g, p = b // 4, b % 4
                if p == 0:
                    U = U2[g % 2]
                    for pp in range(4):
                        nc.sync.dma_start(
                            out=U[32 * pp : 32 * pp + 32, :, 0:D],
                            in_=x_in[4 * g + pp].rearrange("(f c) d -> c f d", c=32),
                        )
                    XT4 = xtp.tile([128, S], F32, name=f"xt4_{g}", tag="xt")
                    nc.vector.transpose(XT4[:, :], U.rearrange("P a c -> P (a c)"))
                    XT4b = xbp.tile([128, S], BF16, name=f"xtb_{g}", tag="xb")
                    nc.vector.tensor_copy(XT4b[:, :], XT4[:, :])
                    st["XT4_new"], st["XT4b"] = XT4, XT4b

                XT4b = st["XT4b"]
                # Q/K projections into one scores-pool pair; single merged copy
                ps_qk = scp.tile([128, 2 * S], F32, name=f"qk_{b}", tag="sc")
                nc.tensor.matmul(
                    ps_qk[:, 0:S], C["wqe"][32 * p : 32 * p + D, :],
                    XT4b[32 * p : 32 * p + D, :],
                    start=True, stop=True, tile_position=(32 * p, 0),
                )
                nc.tensor.matmul(
                    ps_qk[:, S : 2 * S], C["wke"][32 * p : 32 * p + D, :],
                    XT4b[32 * p : 32 * p + D, :],
                    start=True, stop=True, tile_position=(32 * p, 0),
                )
                QKb = qkbp.tile([128, 2 * S], BF16, name=f"qkb_{b}", tag="qkb")
                if b % 2 == 0:
                    nc.scalar.copy(QKb[:, :], ps_qk[:, :])
                else:
                    nc.vector.tensor_copy(QKb[:, :], ps_qk[:, :])

                # V natural layout
                V4ps = mpp.tile([128, 128], F32, name=f"v4_{b}", tag="mp")
                for cch in range(4):
                    nc.tensor.matmul(
                        V4ps[:, 32 * cch : 32 * cch + 32],
                        XT4b[32 * p : 32 * p + D, 128 * cch : 128 * (cch + 1)],
                        C["wve"][32 * p : 32 * p + D, :],
                        start=True, stop=True, tile_position=(32 * p, 0),
                    )
                Vsb = VSB2[b % 2]
                nc.vector.tensor_copy(
                    Vsb.rearrange("P t (h m) -> P t h m", m=32)[:, :, :, 1 : 1 + HD],
                    V4ps.rearrange("P (c x) -> P c x", x=32)[:, :, 0:D].rearrange(
                        "P c (h m) -> P c h m", m=HD
                    ),
                )

                # scores + exp + AV
                E = ep.tile([128, 4, 4, S], BF16, name=f"e_{b}", tag="e")
                UO = uop.tile([128, S], F32, name=f"uo_{b}", tag="uo")
                for t in range(4):
                    for slot in range(2):
                        h0 = 2 * slot
                        SC = scp.tile([128, 2 * S], F32, name=f"sc{b}_{t}_{slot}",
                                      tag="sc")
                        for hh in range(2):
                            h = h0 + hh
                            nc.tensor.matmul(
                                SC[:, S * hh : S * (hh + 1)],
                                QKb[32 * h : 32 * h + HD,
                                    S + 128 * t : S + 128 * (t + 1)],
                                QKb[32 * h : 32 * h + HD, 0:S],
                                start=True, stop=True,
                                tile_position=(32 * h, 0),
                            )
                        edst = E[:, t, h0 : h0 + 2, :]
                        if EXP_ASSIGN[2 * t + slot] == "A":
                            nc.scalar.activation(
                                edst, SC[:, :], AF.Exp, scale=float(SCALE)
                            )
                        else:
                            nc.vector._custom_dve(
                                EXP64, out=edst, in0=SC[:, :], s0=C0EXP
                            )
                    for h in range(4):
                        nc.tensor.matmul(
                            UO[32 * h : 32 * h + 32, :],
                            Vsb[:, t, 32 * h : 32 * h + 32],
                            E[:, t, h, :],
                            start=(t == 0), stop=(t == 3),
                            tile_position=(0, 32 * h),
                            skip_group_check=True,
                        )

                # pipelined tail stages (previous two groups)
                run_tails(g, p)

                # softmax denominator + normalize + Wo
                RR = rrp.tile([128, S], F32, name=f"rr_{b}", tag="rr")
                nc.vector.reciprocal_approx_fast(RR[:, :], UO[:, :])
                RB = rbp.tile([128, S], F32, name=f"rb_{b}", tag="rb")
                nc.vector.stream_shuffle(RB[:, :], RR[:, :], BCAST_MASK)
                OTn = otp.tile([128, S], BF16, name=f"ot_{b}", tag="ot")
                nc.vector.tensor_mul(OTn[:, :], UO[:, :], RB[:, :])
                if p == 0:
                    st["WOPS_new"] = wop.tile([128, S], F32, name=f"wops_{g}",
                                              tag="wops")
                nc.tensor.matmul(
                    st["WOPS_new"][32 * p : 32 * p + 32, :],
                    C["woe"][:, :], OTn[:, :],
                    start=True, stop=True, tile_position=(0, 32 * p),
                    skip_group_check=True,
                )
                if p == 3:
                    st["WOPS"], st["XT4"] = st["WOPS_new"], st["XT4_new"]

            # drain: virtual batches continue the tail schedule
            for vb in range(nb, nb + 8):
                gg, p = vb // 4, vb % 4
                if gg - 1 < ngroups and gg >= 1 and vb < nb + 4:
                    SLOTS[p](gg - 1)
                if gg - 2 < ngroups and gg >= 2:
                    SLOTS[4 + p](gg - 2)
    nc.compile()
    return nc


_NC_CACHE: dict = {}


def _get_nc(nb: int, trivial_affine: bool = True) -> bass.Bass:
    key = (nb, trivial_affine)
    if key not in _NC_CACHE:
        _NC_CACHE[key] = build_nc(nb, trivial_affine)
    return _NC_CACHE[key]


def kernel(x, Wq, Wk, Wv, Wo, W1, W2, g1, b1, g2, b2):
    x = np.asarray(x, np.float32)
    args = [np.asarray(a, np.float32) for a in (Wq, Wk, Wv, Wo, W1, W2, g1, b1, g2, b2)]
    consts = _host_consts(*args)
    g1a, b1a, g2a, b2a = args[6], args[7], args[8], args[9]
    trivial = bool(
        np.all(g1a == 1.0) and np.all(b1a == 0.0)
        and np.all(g2a == 1.0) and np.all(b2a == 0.0)
    )
    nc = _get_nc(NB, trivial)
    in_maps = []
    for c in range(NCORES):
        m = {"x": np.ascontiguousarray(x[c * NB : (c + 1) * NB])}
        m.update(consts)
        in_maps.append(m)
    res = run_bass_kernel_spmd(nc, in_maps, list(range(NCORES)))
    return np.concatenate([r["out"] for r in res.results], axis=0)


# revision 18
# speedup vs baseline: 2.0787x; 1.2512x over previous
"""Trainium2 Bass kernel for nn_Encoder_block (B=128,S=512,D=24,H=4,HD=6,DFF=48).

Pure data parallel over batch: 16 batches/core x 8 cores. Per core, batches
run in 4 groups of 4 banded onto the 128 partitions in T-layout ([d, token],
batch p of a group at partitions 32p..32p+24).

v4 design:
  - softmax exp split across ScalarE (native Exp) and VectorE (EXP64_ANT =
    (1+s/64)^64 fused); [128,1024] per-(t,head-pair) PSUM tiles.
  - scores PSUM pool is 3 deep (6 banks); LN-stats and FFN matmuls borrow
    short-lived tiles from the same pool; V projection reuses the UO bank;
    so the scores pipeline never waits more than one exp.
  - softmax tail (denominator bcast via stream_shuffle + fused 1-Newton
    reciprocal*UO custom op + Wo matmul) is software-pipelined one batch
    behind attention; LN/FFN tails are pipelined over the two following
    groups in 8 slots.
  - everything matmul is bf16; LN affine folded into the gp multiply when
    g==1,b==0 (host-detected).
"""

import os
import sys

import numpy as np

for _p in ("/opt/trn_rl_repo", "/opt/trn_rl_repo/concourse"):
    if os.path.isdir(_p) and _p not in sys.path:
        sys.path.insert(0, _p)

import concourse.bass as bass
import concourse.bacc as bacc
import concourse.mybir as mybir
import concourse.tile as tile
from concourse.bass_utils import run_bass_kernel_spmd

F32 = mybir.dt.float32
BF16 = mybir.dt.bfloat16
AF = mybir.ActivationFunctionType
ALU = mybir.AluOpType

B, S, D = 128, 512, 24
H, HD, DFF = 4, 6, 48
EPS = 1e-5
NCORES = 8
NB = B // NCORES
SCALE = 1.0 / np.sqrt(HD)
EXPN = 64
C0EXP = float(SCALE / EXPN)

# exp engine per (batch parity, 2t+slot): "A"=ScalarE, "D"=VectorE; 9A/7D
EXP_ASSIGN = [
    ["A", "D", "D", "A", "A", "D", "A", "D"],
    ["A", "D", "D", "A", "A", "D", "D", "A"],
]
BCAST_MASK = [0] * 32
# seed constants for the BITWISE_NOT reciprocal (see dve_ops.py)
RC0, RC1 = -0.23549792, 2.0017324


def _register_custom_dve_ops():
    import concourse.dve_ops as dve_ops
    from concourse.dve_spec import (
        Spec, Src0, Src1, One, C0, C1, AluOp, Bin, sq, relu, lower, _has_src1,
    )
    from concourse.dve_uop import DveOpSpec

    if getattr(dve_ops, "_ant_encoder_ops", None) is not None:
        return dve_ops._ant_encoder_ops

    def _exp64_ref(in0, in1, s0, s1, imm2):
        return ((1.0 + in0.astype(np.float32) * s0) ** 64).astype(np.float32)

    b = One + Src0 * C0
    for _ in range(6):
        b = sq(b)

    _not = Bin(AluOp.BITWISE_NOT, Src1, Src1)
    _y0 = _not * C0
    _y1 = _y0 * (C1 - Src1 * _y0)

    def _recip_mul_ref(in0, in1, s0, s1, imm2):
        nx = (~in1.view(np.int32)).view(np.float32)
        y0 = nx * s0
        y1 = y0 * (s1 - in1 * y0)
        return (in0.astype(np.float32) * y1).astype(np.float32)

    specs = {
        "EXP64_ANT": Spec(body=b, reference=_exp64_ref),
        "RELU_ADD_ANT": Spec(
            body=relu(Src0) + Src1,
            reference=lambda in0, in1, s0, s1, imm2: np.maximum(
                np.nan_to_num(in0.astype(np.float32), nan=0.0), 0
            )
            + in1,
        ),
        "SUBSQ_ANT": Spec(
            body=Src0 - sq(Src1),
            reference=lambda in0, in1, s0, s1, imm2: in0.astype(np.float32)
            - in1.astype(np.float32) * in1.astype(np.float32),
        ),
        "RECIP_MUL_ANT": Spec(body=Src0 * _y1, reference=_recip_mul_ref),
    }
    ops = {}
    for name, spec in specs.items():
        shas = {}
        for ver in ("v3", "v4"):
            tmp = DveOpSpec(
                name=name, opcode=0, uops=lower(spec, ver=ver), rd1_en=_has_src1(spec)
            )
            shas[ver] = tmp.sha(ver)
        op = dve_ops.DveOp(name, spec, subdim=False, uops_sha=shas)
        dve_ops.OPS.append(op)
        ops[name] = op
    dve_ops._SUB_OPCODE_FOR_NAME.clear()
    dve_ops._SUB_OPCODE_FOR_NAME.update(
        {op.name: dve_ops._CUSTOM_DVE_ROW_BASE + i for i, op in enumerate(dve_ops.OPS)}
    )
    assert max(dve_ops._SUB_OPCODE_FOR_NAME.values()) < 0x20
    dve_ops.CUSTOM_DVE_SPECS.update({n: s for n, s in specs.items()})
    dve_ops._ant_encoder_ops = ops
    return ops


def _host_consts(Wq, Wk, Wv, Wo, W1, W2, g1, b1, g2, b2):
    import ml_dtypes

    bf = ml_dtypes.bfloat16
    c = {}
    wqe = np.zeros((128, 128), np.float32)
    wke = np.zeros((128, 128), np.float32)
    for p in range(4):
        for h in range(H):
            for j in range(HD):
                wqe[32 * p : 32 * p + D, 32 * h + j] = Wq[6 * h + j, :]
                wke[32 * p : 32 * p + D, 32 * h + j] = Wk[6 * h + j, :]
    c["wqe"] = wqe.astype(bf)
    c["wke"] = wke.astype(bf)

    wve = np.zeros((128, 32), np.float32)
    for p in range(4):
        for j in range(D):
            wve[32 * p : 32 * p + D, j] = Wv[j, :]
    c["wve"] = wve.astype(bf)

    woe = np.zeros((128, 32), np.float32)
    for h in range(H):
        for j in range(HD):
            woe[32 * h + 1 + j, 0:D] = Wo[:, 6 * h + j]
    c["woe"] = woe.astype(bf)

    cb1 = np.zeros((128, 128), np.float32)
    cb2 = np.zeros((128, 128), np.float32)
    for p in range(4):
        cb1[32 * p : 32 * p + D, 32 * p] = -1.0 / D
        cb2[32 * p : 32 * p + D, 32 * p] = 1.0 / D
    c["cb1"] = cb1.astype(bf)
    c["cb2"] = cb2.astype(bf)

    w1e = np.zeros((128, 4 * 64), np.float32)
    for p in range(4):
        w1e[32 * p : 32 * p + D, 64 * p : 64 * p + DFF] = W1.T
    c["w1e"] = w1e.astype(bf)

    w2e = np.zeros((128, 2 * 32), np.float32)
    w2e[0:DFF, 0:D] = W2.T
    w2e[64 : 64 + DFF, 32 : 32 + D] = W2.T
    c["w2e"] = w2e.astype(bf)

    gb = np.zeros((128, 4), np.float32)
    for p in range(4):
        gb[32 * p : 32 * p + D, 0] = g1
        gb[32 * p : 32 * p + D, 1] = b1
        gb[32 * p : 32 * p + D, 2] = g2
        gb[32 * p : 32 * p + D, 3] = b2
    c["gb"] = gb
    return c


CONST_SHAPES = {
    "wqe": ((128, 128), BF16),
    "wke": ((128, 128), BF16),
    "wve": ((128, 32), BF16),
    "woe": ((128, 32), BF16),
    "cb1": ((128, 128), BF16),
    "cb2": ((128, 128), BF16),
    "w1e": ((128, 4 * 64), BF16),
    "w2e": ((128, 2 * 32), BF16),
    "gb": ((128, 4), F32),
}


def _pin_act_tables():
    import concourse.bacc as _bacc

    if getattr(_bacc, "_act_tables_pinned", False):
        return
    _orig = _bacc.get_activation_tables

    def _patched(arch):
        tables = dict(_orig(arch))
        keep = "natural_log_exp_and_others"
        for name in list(tables):
            if name != keep and (AF.Exp in tables[name] or AF.Ln in tables[name]):
                tables[name] = set()
        return tables

    _bacc.get_activation_tables = _patched
    _bacc._act_tables_pinned = True


def build_nc(nb: int = NB, trivial_affine: bool = True) -> bass.Bass:
    _pin_act_tables()
    OPS = _register_custom_dve_ops()
    EXP64, RELU_ADD = OPS["EXP64_ANT"], OPS["RELU_ADD_ANT"]
    SUBSQ, RECIP_MUL = OPS["SUBSQ_ANT"], OPS["RECIP_MUL_ANT"]
    ngroups = nb // 4
    nc = bacc.Bacc()
    x_in = nc.dram_tensor("x", [nb, S, D], F32, kind="ExternalInput")
    out = nc.dram_tensor("out", [nb, S, D], F32, kind="ExternalOutput")
    cin = {
        k: nc.dram_tensor(k, list(sh), dt, kind="ExternalInput")
        for k, (sh, dt) in CONST_SHAPES.items()
    }

    with tile.TileContext(nc) as tc:
        import contextlib

        ctx = contextlib.ExitStack()
        with ctx:
            constp = ctx.enter_context(tc.tile_pool(name="consts", bufs=1))
            persp = ctx.enter_context(tc.tile_pool(name="pers", bufs=1))
            xtp = ctx.enter_context(tc.tile_pool(name="xt", bufs=2))
            xbp = ctx.enter_context(tc.tile_pool(name="xb", bufs=2))
            qkbp = ctx.enter_context(tc.tile_pool(name="qkb", bufs=2))
            ep = ctx.enter_context(tc.tile_pool(name="e", bufs=2))
            dbp = ctx.enter_context(tc.tile_pool(name="db", bufs=2))
            otp = ctx.enter_context(tc.tile_pool(name="ot", bufs=2))
            y1p = ctx.enter_context(tc.tile_pool(name="y1", bufs=2))
            ybp = ctx.enter_context(tc.tile_pool(name="yb", bufs=4))
            smp = ctx.enter_context(tc.tile_pool(name="sm", bufs=4))
            bcp = ctx.enter_context(tc.tile_pool(name="bc", bufs=4))
            x1p = ctx.enter_context(tc.tile_pool(name="x1", bufs=2))
            hsp = ctx.enter_context(tc.tile_pool(name="hs", bufs=4))
            fsp = ctx.enter_context(tc.tile_pool(name="fs", bufs=2))
            y2p = ctx.enter_context(tc.tile_pool(name="y2", bufs=2))
            # PSUM: scores/qk/stats/ffn 3x[128,1024]=6 banks, UO 1, WOPS 1
            scp = ctx.enter_context(tc.tile_pool(name="sc", bufs=3, space="PSUM"))
            uop = ctx.enter_context(tc.tile_pool(name="uo", bufs=1, space="PSUM"))
            wop = ctx.enter_context(tc.tile_pool(name="wo", bufs=1, space="PSUM"))

            C = {}
            for k, (sh, dt) in CONST_SHAPES.items():
                t = constp.tile(list(sh), dt, name=f"c_{k}")
                nc.sync.dma_start(out=t, in_=cin[k][:, :])
                C[k] = t
            eps_t = constp.tile([128, 1], F32, name="c_eps")
            nc.vector.memset(eps_t, EPS)

            U2, VSB2 = [], []
            for i in range(2):
                u = persp.tile([128, 16, 32], F32, name=f"u{i}")
                nc.vector.memset(u[:, :, D:32], 0.0)
                U2.append(u)
                v = persp.tile([128, 4, 128], BF16, name=f"vsb{i}")
                nc.gpsimd.memset(v[:, :, :], 1.0)
                VSB2.append(v)

            def bcast_dma(dst, src):
                for h in range(4):
                    row = src[32 * h : 32 * h + 1, :]
                    src_b = bass.AP(
                        tensor=row.tensor,
                        offset=row.offset,
                        ap=[list(row.ap[0]), [0, 32]]
                        + [list(x) for x in row.ap[1:]],
                    )
                    nc.sync.dma_start(
                        out=dst[32 * h : 32 * h + 32, :].rearrange(
                            "p (x q) -> p x q", x=1
                        ),
                        in_=src_b,
                    )

            st = {}

            # ---------------- tail pipeline (8 slots / 2 groups) -------------
            def ln_slotA(Y, tag):
                Yb = ybp.tile([128, S], BF16, name=f"yb_{tag}", tag="yb")
                nc.vector.tensor_copy(Yb[:, :], Y[:, :])
                YSQ = ybp.tile([128, S], BF16, name=f"ysq_{tag}", tag="yb")
                nc.gpsimd.tensor_mul(YSQ[:, :], Y[:, :], Y[:, :])
                return Yb, YSQ

            def ln_slotB(YbQ, tag):
                Yb, YSQ = YbQ
                mps = scp.tile([128, 2 * S], F32, name=f"mps_{tag}", tag="sc")
                nc.tensor.matmul(
                    mps[:, 0:S], C["cb1"][:, :], Yb[:, :],
                    start=True, stop=True, tile_position=(0, 0),
                )
                nc.tensor.matmul(
                    mps[:, S : 2 * S], C["cb2"][:, :], YSQ[:, :],
                    start=True, stop=True, tile_position=(0, 0),
                )
                STT = smp.tile([128, 2 * S], BF16, name=f"stt_{tag}", tag="sm")
                nc.scalar.copy(STT[:, 0:S], mps[:, 0:S])
                VAR = smp.tile([128, S], F32, name=f"var_{tag}", tag="var")
                nc.vector._custom_dve(
                    SUBSQ, out=VAR[:, :], in0=mps[:, S : 2 * S], in1=STT[:, 0:S]
                )
                LNV = smp.tile([128, S], F32, name=f"lnv_{tag}", tag="var")
                nc.scalar.activation(LNV[:, :], VAR[:, :], AF.Ln, bias=eps_t[:, :])
                nc.scalar.activation(STT[:, S : 2 * S], LNV[:, :], AF.Exp, scale=-0.5)
                return STT

            def ln_slotC(Y, STT, OUT, gcol, bcol, tag):
                BC = bcp.tile([128, 2 * S], BF16, name=f"bc_{tag}", tag="bc")
                bcast_dma(BC, STT)
                nc.gpsimd.tensor_add(Y[:, :], Y[:, :], BC[:, 0:S])
                if trivial_affine:
                    nc.gpsimd.tensor_mul(OUT[:, :], Y[:, :], BC[:, S : 2 * S])
                else:
                    nc.gpsimd.tensor_mul(Y[:, :], Y[:, :], BC[:, S : 2 * S])
                    nc.vector.tensor_scalar(
                        OUT[:, :], Y[:, :], gcol, bcol, op0=ALU.mult, op1=ALU.add
                    )

            def slot0(g):
                Y1 = y1p.tile([128, S], F32, name=f"y1_{g}", tag="y1")
                nc.vector.tensor_add(Y1[:, :], st["WOPS"][:, :], st["XT4"][:, :])
                st["Y1"] = Y1
                st["L1A"] = ln_slotA(Y1, f"l1g{g}")

            def slot1(g):
                st["L1B"] = ln_slotB(st["L1A"], f"l1g{g}")

            def slot2(g):
                X1 = x1p.tile([128, S], BF16, name=f"x1_{g}", tag="x1")
                ln_slotC(st["Y1"], st["L1B"], X1,
                         C["gb"][:, 0:1], C["gb"][:, 1:2], f"l1g{g}")
                st["X1"] = X1

            def slot3(g):
                X1 = st["X1"]
                hps = scp.tile([128, 2 * S], F32, name=f"hps_{g}", tag="sc")
                for pair in range(2):
                    for j in range(2):
                        p4 = 2 * pair + j
                        nc.tensor.matmul(
                            hps[64 * j : 64 * j + 64, S * pair : S * (pair + 1)],
                            C["w1e"][:, 64 * p4 : 64 * (p4 + 1)],
                            X1[:, :],
                            start=True, stop=True, tile_position=(0, 64 * j),
                            skip_group_check=True,
                        )
                HS = hsp.tile([128, 2 * S], BF16, name=f"hs_{g}", tag="hs")
                nc.scalar.activation(HS[:, :], hps[:, :], AF.Relu)
                f4t = scp.tile([128, 2 * S], F32, name=f"f4_{g}", tag="sc")
                F4 = f4t[:, 0:S]
                for pair in range(2):
                    for j in range(2):
                        p4 = 2 * pair + j
                        nc.tensor.matmul(
                            F4[32 * p4 : 32 * p4 + 32, :],
                            C["w2e"][:, 32 * j : 32 * (j + 1)],
                            HS[:, S * pair : S * (pair + 1)],
                            start=True, stop=True, tile_position=(0, 32 * p4),
                            skip_group_check=True,
                        )
                FS = fsp.tile([128, S], F32, name=f"fs_{g}", tag="fs")
                nc.vector._custom_dve(
                    RELU_ADD, out=FS[:, :], in0=F4, in1=X1[:, :]
                )
                st["FS"] = FS

            def slot4(g):
                st["L2A"] = ln_slotA(st["FS"], f"l2g{g}")

            def slot5(g):
                st["L2B"] = ln_slotB(st["L2A"], f"l2g{g}")

            def slot6(g):
                Y2N = y2p.tile([128, S], F32, name=f"y2n_{g}", tag="y2n")
                ln_slotC(st["FS"], st["L2B"], Y2N,
                         C["gb"][:, 2:3], C["gb"][:, 3:4], f"l2g{g}")
                st["Y2N"] = Y2N

            def slot7(g):
                Y2T = y2p.tile([128, S], F32, name=f"y2t_{g}", tag="y2t")
                nc.vector.transpose(Y2T[:, :], st["Y2N"][:, :])
                for pp in range(4):
                    nc.sync.dma_start(
                        out=out[4 * g + pp].rearrange("(f r) d -> r f d", r=32),
                        in_=Y2T[32 * pp : 32 * pp + 32, :].rearrange(
                            "r (f c) -> r f c", c=32
                        )[:, :, 0:D],
                    )

            SLOTS = [slot0, slot1, slot2, slot3, slot4, slot5, slot6, slot7]

            def run_tails(gg, p):
                if gg >= 1:
                    SLOTS[p](gg - 1)
                if gg >= 2:
                    SLOTS[4 + p](gg - 2)

            def softmax_tail(bprev):
                """denominator bcast + normalize + Wo for batch bprev."""
                UO = st["UO_prev"]
                DB = dbp.tile([128, S], F32, name=f"db_{bprev}", tag="db")
                nc.vector.stream_shuffle(DB[:, :], UO[:, :], BCAST_MASK)
                OTn = otp.tile([128, S], BF16, name=f"ot_{bprev}", tag="ot")
                nc.vector._custom_dve(
                    RECIP_MUL, out=OTn[:, :], in0=UO[:, :], in1=DB[:, :],
                    s0=RC0, s1=RC1,
                )
                st["OTn_prev"] = OTn

            def wo_mm(bprev):
                gprev, pprev = bprev // 4, bprev % 4
                if pprev == 0:
                    st["WOPS_new"] = wop.tile(
                        [128, S], F32, name=f"wops_{gprev}", tag="wops"
                    )
                nc.tensor.matmul(
                    st["WOPS_new"][32 * pprev : 32 * pprev + 32, :],
                    C["woe"][:, :], st["OTn_prev"][:, :],
                    start=True, stop=True, tile_position=(0, 32 * pprev),
                    skip_group_check=True,
                )
                if pprev == 3:
                    st["WOPS"], st["XT4"] = st["WOPS_new"], st["XT4_keep"]

            # --------------------------- main loop ---------------------------
            for b in range(nb):
                g, p = b // 4, b % 4
                if p == 0:
                    U = U2[g % 2]
                    for pp in range(4):
                        nc.sync.dma_start(
                            out=U[32 * pp : 32 * pp + 32, :, 0:D],
                            in_=x_in[4 * g + pp].rearrange("(f c) d -> c f d", c=32),
                        )
                    XT4 = xtp.tile([128, S], F32, name=f"xt4_{g}", tag="xt")
                    nc.vector.transpose(XT4[:, :], U.rearrange("P a c -> P (a c)"))
                    XT4b = xbp.tile([128, S], BF16, name=f"xtb_{g}", tag="xb")
                    nc.vector.tensor_copy(XT4b[:, :], XT4[:, :])
                    st["XT4_new"], st["XT4b"] = XT4, XT4b

                XT4b = st["XT4b"]
                # Q/K projections -> one merged bf16 copy
                ps_qk = scp.tile([128, 2 * S], F32, name=f"qk_{b}", tag="sc")
                nc.tensor.matmul(
                    ps_qk[:, 0:S], C["wqe"][32 * p : 32 * p + D, :],
                    XT4b[32 * p : 32 * p + D, :],
                    start=True, stop=True, tile_position=(32 * p, 0),
                )
                nc.tensor.matmul(
                    ps_qk[:, S : 2 * S], C["wke"][32 * p : 32 * p + D, :],
                    XT4b[32 * p : 32 * p + D, :],
                    start=True, stop=True, tile_position=(32 * p, 0),
                )
                # previous batch's AV last chunk + softmax tail fills the gap
                if b > 0:
                    Ep, UOp, VSBp = st["E_prev"], st["UO_prev"], st["VSB_prev"]
                    for h in range(4):
                        nc.tensor.matmul(
                            UOp[32 * h : 32 * h + 32, :],
                            VSBp[:, 3, 32 * h : 32 * h + 32],
                            Ep[:, 3, h, :],
                            start=False, stop=True,
                            tile_position=(0, 32 * h),
                            skip_group_check=True,
                        )
                QKb = qkbp.tile([128, 2 * S], BF16, name=f"qkb_{b}", tag="qkb")
                if b % 2 == 0:
                    nc.scalar.copy(QKb[:, :], ps_qk[:, :])
                else:
                    nc.vector.tensor_copy(QKb[:, :], ps_qk[:, :])
                if b > 0:
                    softmax_tail(b - 1)

                # scores + exp + AV (av(t) emitted after sc(t+1))
                E = ep.tile([128, 4, 4, S], BF16, name=f"e_{b}", tag="e")

                def sc_t(t):
                    for slot in range(2):
                        h0 = 2 * slot
                        SC = scp.tile([128, 2 * S], F32,
                                      name=f"sc{b}_{t}_{slot}", tag="sc")
                        for hh in range(2):
                            h = h0 + hh
                            nc.tensor.matmul(
                                SC[:, S * hh : S * (hh + 1)],
                                QKb[32 * h : 32 * h + HD,
                                    S + 128 * t : S + 128 * (t + 1)],
                                QKb[32 * h : 32 * h + HD, 0:S],
                                start=True, stop=True,
                                tile_position=(32 * h, 0),
                            )
                        edst = E[:, t, h0 : h0 + 2, :]
                        if EXP_ASSIGN[b % 2][2 * t + slot] == "A":
                            nc.scalar.activation(
                                edst, SC[:, :], AF.Exp, scale=float(SCALE)
                            )
                        else:
                            nc.vector._custom_dve(
                                EXP64, out=edst, in0=SC[:, :], s0=C0EXP
                            )

                def av_t(t):
                    for h in range(4):
                        nc.tensor.matmul(
                            UO[32 * h : 32 * h + 32, :],
                            Vsb[:, t, 32 * h : 32 * h + 32],
                            E[:, t, h, :],
                            start=(t == 0), stop=False,
                            tile_position=(0, 32 * h),
                            skip_group_check=True,
                        )

                sc_t(0)
                # V natural layout into the UO bank (before av(0) clears it)
                UO = uop.tile([128, S], F32, name=f"uo_{b}", tag="uo")
                for cch in range(4):
                    nc.tensor.matmul(
                        UO[:, 32 * cch : 32 * cch + 32],
                        XT4b[32 * p : 32 * p + D, 128 * cch : 128 * (cch + 1)],
                        C["wve"][32 * p : 32 * p + D, :],
                        start=True, stop=True, tile_position=(32 * p, 0),
                    )
                Vsb = VSB2[b % 2]
                nc.vector.tensor_copy(
                    Vsb.rearrange("P t (h m) -> P t h m", m=32)[:, :, :, 1 : 1 + HD],
                    UO.rearrange("P (c x) -> P c x", x=32)[:, 0:4, 0:D].rearrange(
                        "P c (h m) -> P c h m", m=HD
                    ),
                )
                sc_t(1)
                if b > 0:
                    wo_mm(b - 1)
                av_t(0)
                sc_t(2)
                av_t(1)
                sc_t(3)
                av_t(2)
                # av(3) is emitted at the start of the next batch

                run_tails(g, p)
                st["E_prev"], st["UO_prev"], st["VSB_prev"] = E, UO, Vsb
                if p == 0:
                    st["XT4_keep"] = st["XT4_new"]

            # drain: last batch's av(3) + softmax tail + wo, then tail slots
            b = nb
            Ep, UOp, VSBp = st["E_prev"], st["UO_prev"], st["VSB_prev"]
            for h in range(4):
                nc.tensor.matmul(
                    UOp[32 * h : 32 * h + 32, :],
                    VSBp[:, 3, 32 * h : 32 * h + 32],
                    Ep[:, 3, h, :],
                    start=False, stop=True,
                    tile_position=(0, 32 * h),
                    skip_group_check=True,
                )
            softmax_tail(nb - 1)
            wo_mm(nb - 1)
            for vb in range(nb, nb + 8):
                gg, pp = vb // 4, vb % 4
                if gg >= 1 and gg - 1 < ngroups and vb < nb + 4:
                    SLOTS[pp](gg - 1)
                if gg >= 2 and gg - 2 < ngroups:
                    SLOTS[4 + pp](gg - 2)
    nc.compile()
    return nc


_NC_CACHE: dict = {}


def _get_nc(nb: int, trivial_affine: bool = True) -> bass.Bass:
    key = (nb, trivial_affine)
    if key not in _NC_CACHE:
        _NC_CACHE[key] = build_nc(nb, trivial_affine)
    return _NC_CACHE[key]


def kernel(x, Wq, Wk, Wv, Wo, W1, W2, g1, b1, g2, b2):
    x = np.asarray(x, np.float32)
    args = [np.asarray(a, np.float32) for a in (Wq, Wk, Wv, Wo, W1, W2, g1, b1, g2, b2)]
    consts = _host_consts(*args)
    g1a, b1a, g2a, b2a = args[6], args[7], args[8], args[9]
    trivial = bool(
        np.all(g1a == 1.0) and np.all(b1a == 0.0)
        and np.all(g2a == 1.0) and np.all(b2a == 0.0)
    )
    nc = _get_nc(NB, trivial)
    in_maps = []
    for c in range(NCORES):
        m = {"x": np.ascontiguousarray(x[c * NB : (c + 1) * NB])}
        m.update(consts)
        in_maps.append(m)
    res = run_bass_kernel_spmd(nc, in_maps, list(range(NCORES)))
    return np.concatenate([r["out"] for r in res.results], axis=0)
